# revision 33
# baseline (speedup 1.0000x reference)
"""AWQ int4 GEMM (M=1024, K=4096, N=11008, group_size=128) on 8 TRN2 NeuronCores.

Column-parallel tensor sharding (vLLM-style): qweight/qzeros/scales split
along N across the 8 cores, activations replicated, outputs concatenated.
Each core is fully independent (no collectives).

v3 structure — three N-phases so the PE never waits on dequant DMA:
  The dequantized weight tile W [128, KT, 1408] stays fully resident in
  SBUF, and the PE sweeps it in three column passes, each a plain
  kt(outer) x m(inner) accumulation into 8 PSUM banks (one per m-tile):
    phase 0: cols    0: 512  = perm-cols 0:480  ++ 32 one-hot columns
    phase 1: cols  512:1024  = perm-cols 480:992
    phase 2: cols 1024:1408  = perm-cols 992:1376
  The scale broadcast (128x-replicated rows, 11.3 MB) is split into
  per-phase column slices stored phase-sliced in DRAM so every transfer is
  single-segment; dequant for phase p+1 prefetches while the PE consumes
  phase p.  qweight is loaded once (22KB/partition, 4 big-descriptor
  chunks) and stays resident; x is partition-major in DRAM so its loads
  are 8KB-descriptor chunks.  All heavy streams are near peak per-queue
  DMA efficiency, leaving headroom for the prefetch.

  The one-hot columns compute xsum on the PE for free: column 480+g of
  k-tile kt's rhs is ones iff g==kt, so after phase 0 psum cols 480:512
  hold xsum[m-row, g].  Per m-tile that block is copied to bf16 with
  scale=-1 (free negation on the Act engine), transposed by the DMA XBAR,
  and used as the stationary operand of the per-phase zero-point
  correction matmul: out += (-xsum_g) @ (z_g * s_g).

  Outputs are written per (m-tile, phase) straight from PSUM->SBUF->DRAM
  in pair-block-permuted column order; the host undoes the permutation
  (pure reindex, no FLOPs), which removes the on-device unpermute copies
  and the full-width output staging tile.
"""

import os
import sys
import types

sys.path.insert(0, "/opt/trn_rl_repo")

import numpy as np
import ml_dtypes

import concourse.bass as bass
import concourse.bass_isa as bass_isa
import concourse.mybir as mybir
import concourse.tile as tile
import bass_rust as _br
from concourse.vector_clock import ScopedClock
from concourse.bass_utils import run_bass_kernel_spmd


# ---------------------------------------------------------------------------
# Walrus workaround: this toolchain rejects >1 sem wait per instruction
# (2 for InstEventSemaphore).  Tile's sem assigner can emit more; split the
# excess onto no-ops placed immediately before on the same engine.
# ---------------------------------------------------------------------------
_orig_lower = tile.TileContext._lower_ordered_insts
_wsplit_counter = [0]


def _split_waits_in_place(nc, insts):
    new_list = []
    for inst in insts:
        si = inst.sync_info
        cap = 2 if isinstance(inst, mybir.InstEventSemaphore) else 1
        if si is not None and len(si.on_wait) > cap:
            waits = list(si.on_wait)
            extra, keep = waits[:-cap], waits[-cap:]
            for w in extra:
                _wsplit_counter[0] += 1
                nop = mybir.InstNoOp(
                    name=f"wsplit-{_wsplit_counter[0]}",
                    engine=inst.engine,
                    sync_info=mybir.SyncInfo(on_wait=[w], on_update=[]),
                    bass_nofuse=True,
                )
                nc.register_instruction(nop)
                new_list.append(nop)
            inst.sync_info = mybir.SyncInfo(on_wait=keep, on_update=list(si.on_update))
        new_list.append(inst)
    insts[:] = new_list


def _dedup_ldweights_in_place(nc, insts):
    """Drop InstLdweights identical to the still-loaded stationary operand."""
    last_sig = None
    out = []
    for inst in insts:
        nm = inst.__class__.__name__
        if nm == "InstLdweights":
            sig = repr(inst.ins[0])
            if sig == last_sig:
                si = inst.sync_info
                if si is not None and (si.on_wait or si.on_update):
                    _wsplit_counter[0] += 1
                    nop = mybir.InstNoOp(
                        name=f"ldwkill-{_wsplit_counter[0]}",
                        engine=inst.engine,
                        sync_info=mybir.SyncInfo(
                            on_wait=list(si.on_wait),
                            on_update=list(si.on_update),
                        ),
                        bass_nofuse=True,
                    )
                    nc.register_instruction(nop)
                    out.append(nop)
                continue
            last_sig = sig
        elif nm != "InstMatmult" and inst.engine == mybir.EngineType.PE:
            last_sig = None
        out.append(inst)
    insts[:] = out


def _patched_lower(self, ordered):
    for insts in ordered.values():
        _dedup_ldweights_in_place(self.nc, insts)
        _split_waits_in_place(self.nc, insts)
    return _orig_lower(self, ordered)


def _patched_drain_and_barrier(self, tick_clock, wait_clock):
    nc = self.nc
    drain_inst = nc.sync.drain()
    wait_clock.add_sem_waits(
        drain_inst.ins, ScopedClock({None: tick_clock.global_clock})
    )
    si = drain_inst.ins.sync_info
    if si is not None and len(si.on_wait) > 1:
        waits = list(si.on_wait)
        drain_inst.ins.sync_info = _br.SyncInfo(
            on_wait=[waits[0]], on_update=list(si.on_update)
        )
        for w in waits[1:]:
            extra = nc.sync.drain()
            extra.ins.sync_info = _br.SyncInfo(on_wait=[w], on_update=[])
    nc.all_engine_barrier()
    assert self.sems is not None
    popped = nc._tile_sem_poison_stack.pop()
    assert popped is self._sem_poison
    nc.clear_and_free_semaphores(list(self.sems.allocated().values()))
    nc.all_engine_barrier()


tile.TileContext._lower_ordered_insts = _patched_lower
tile.TileContext._drain_and_barrier = _patched_drain_and_barrier

# ---------------------------------------------------------------------------
# NTFF profiling hook shim (only used when tracing is requested).
# ---------------------------------------------------------------------------
def _install_ntff_shim():
    if "antenv.axon_hooks" in sys.modules:
        return
    try:
        from trn_agent_boot.trn_boot import _ntff_profile_via_ctypes

        hook = _ntff_profile_via_ctypes("/opt/axon/libaxon_pjrt.so")
    except Exception:
        hook = None
    m = types.ModuleType("antenv.axon_hooks")
    m.get_axon_ntff_profile_hook = lambda: hook
    m.set_axon_ntff_profile_hook = lambda h: None
    import antenv  # noqa: F401

    sys.modules["antenv.axon_hooks"] = m


# ---------------------------------------------------------------------------
# Problem shape (hardcoded per contract)
# ---------------------------------------------------------------------------
M, K, N_TOTAL = 1024, 4096, 11008
NCORES = 8
N_LOC = N_TOTAL // NCORES  # 1376 unpacked columns per core
NP = N_LOC // 8            # 172 packed int32 columns per core
G = 32                     # scale/zero groups (group_size 128 == k-tile)
KT = K // 128              # 32 k-tiles
MT = M // 128              # 8 m-tiles
NB = 4                     # pair-blocks per core (one per unpack shift)
BW = N_LOC // NB           # 344 perm columns per pair-block

PAIR_SHIFTS = (0, 4, 8, 12)
SREP = 128  # scale-row replication factor in DRAM
PAIR_MASK = 0x000F000F

F32 = mybir.dt.float32
BF16 = mybir.dt.bfloat16
I32 = mybir.dt.int32
I16 = mybir.dt.int16

# Column phases: (perm_lo, perm_hi, W col offset, matmul width)
NEXT = N_LOC + 32          # 1408 = W row width
PH = (
    (0, 480, 0, 512),      # + one-hot cols 480:512 of W
    (480, 992, 512, 512),
    (992, 1376, 1024, 384),
)
# packed-column pieces (t, c0, c1) per phase: perm cols [344t+2c0, 344t+2c1)
PH_PIECES = (
    ((0, 0, 172), (1, 0, 68)),
    ((1, 68, 172), (2, 0, 152)),
    ((2, 152, 172), (3, 0, 172)),
)
# modeled-timeline pins (ms) for phase 1/2 dequant prefetch
PH_PIN = (None, 0.015, 0.085)
XCH = 4                    # k-tiles per x load chunk
QCH = 8                    # k-tiles per qweight load chunk

# inverse permutation: orig col o -> perm col
_INV = np.array(
    [344 * ((o % 8) // 2) + 2 * (o // 8) + (o % 2) for o in range(N_LOC)],
    dtype=np.int64,
)

# debug switches
_NO_CORR = os.environ.get("AWQ_NO_CORR", "0") == "1"   # skip zero-point corr

LAST_EXEC_NS = None
LAST_TRACE = None

_cached_nc = None


def _build():
    nc = bass.Bass()
    # x partition-major: [128, KT, M], partition p holds rows {kt*128+p}
    xt_d = nc.declare_dram_parameter("xt", [128, KT, M], BF16, isOutput=False)
    qw_d = nc.declare_dram_parameter("qw", [128, KT, NP], I32, isOutput=False)
    # per-phase scale slices, replica-major [SREP, G, w] (perm order)
    sp_d = [
        nc.declare_dram_parameter(f"sp{p}", [SREP, G, PH[p][1] - PH[p][0]],
                                  BF16, isOutput=False)
        for p in range(3)
    ]
    qz_d = nc.declare_dram_parameter("qz", [G, NP], I32, isOutput=False)
    oh_d = nc.declare_dram_parameter("oh", [128, G, G], BF16, isOutput=False)
    # output in PERM column order; host undoes the permutation
    out_d = nc.declare_dram_parameter("out", [M, N_LOC], BF16, isOutput=True)

    AND = mybir.AluOpType.bitwise_and
    LSR = mybir.AluOpType.logical_shift_right
    MUL = mybir.AluOpType.mult

    with tile.TileContext(nc) as tc:
        from contextlib import ExitStack

        with ExitStack() as ctx:
            big = ctx.enter_context(tc.tile_pool(name="big", bufs=1))
            xT = big.tile([128, KT, M], BF16)        # x (k on partitions)
            W = big.tile([128, KT, NEXT], BF16)      # dequant + one-hot cols
            qw_sb = big.tile([128, KT, NP], I32)     # packed qweight, resident

            consts = ctx.enter_context(tc.tile_pool(name="consts", bufs=1))
            qz_sb = consts.tile([G, NP], I32)
            znib = consts.tile([G, NB, NP], I32)
            sps = [consts.tile([G, PH[p][1] - PH[p][0]], BF16, name=f"sps{p}")
                   for p in range(3)]
            Bs = [consts.tile([G, PH[p][1] - PH[p][0]], BF16, name=f"Bs{p}")
                  for p in range(3)]                 # +(z*s) per phase slice
            xsp = consts.tile([128, MT, 128], BF16)  # -xsum staging (padded)
            xsT = consts.tile([128, MT, 128], BF16)  # transposed -xsum per m

            wprep = ctx.enter_context(tc.tile_pool(name="wprep", bufs=2))
            opool = ctx.enter_context(tc.tile_pool(name="oout", bufs=3))

            # priming: first qweight chunk + first scale slice lead the
            # scalar ring; one-hot identity + first x chunk lead sync.
            def qw_load_range(k0, k1):
                nc.scalar.dma_start(
                    out=qw_sb[:, k0:k1, :],
                    in_=qw_d[:, k0:k1, :],
                )

            def x_load(c):
                nc.sync.dma_start(
                    out=xT[:, c * XCH:(c + 1) * XCH, :],
                    in_=xt_d[:, c * XCH:(c + 1) * XCH, :],
                )

            def dequant_pair(ph, j, s2_eng=None):
                plo, phi, wlo, _ = PH[ph]
                w = phi - plo
                pieces = PH_PIECES[ph]
                pw = sum(c1 - c0 for _, c0, c1 in pieces)
                kt = 2 * j
                nib = wprep.tile([128, 2, 256], I32, name="nib", tag="nib",
                                 bufs=2)
                off = 0
                for t, c0, c1 in pieces:
                    nc.vector.tensor_scalar(
                        out=nib[:, :, off:off + (c1 - c0)],
                        in0=qw_sb[:, kt:kt + 2, c0:c1],
                        scalar1=PAIR_SHIFTS[t], scalar2=PAIR_MASK,
                        op0=LSR, op1=AND,
                    )
                    off += c1 - c0
                # scale rows kt, kt+1 of this phase's slice: contiguous
                # single-segment 2*w*2B run per partition (own replica)
                s2 = wprep.tile([128, 2, 512], BF16, name="s2", tag="sbc",
                                bufs=6)
                rep = sp_d[ph][0]  # [G, w] view of replica 0
                (s2_eng or nc.sync).dma_start(
                    out=s2[:, :, 0:w],
                    in_=bass.AP(
                        tensor=rep.tensor,
                        offset=rep.offset + kt * w,
                        ap=[[G * w, 128], [w, 2], [1, w]],
                    ),
                )
                nib16 = nib.bitcast(I16)  # [128, 2, 512]
                nc.vector.tensor_tensor(
                    out=W[:, kt:kt + 2, wlo:wlo + 2 * pw],
                    in0=nib16[:, :, 0:2 * pw],
                    in1=s2[:, :, 0:w],
                    op=MUL,
                )

            def zb_prep():
                for t in range(NB):
                    nc.vector.tensor_scalar(
                        out=znib[:, t, :], in0=qz_sb,
                        scalar1=PAIR_SHIFTS[t], scalar2=PAIR_MASK,
                        op0=LSR, op1=AND,
                    )
                z16 = znib.bitcast(I16).rearrange("p a b -> p (a b)")
                for p in range(3):
                    plo, phi = PH[p][0], PH[p][1]
                    nc.vector.tensor_tensor(
                        out=Bs[p], in0=z16[:, plo:phi], in1=sps[p], op=MUL
                    )

            # one-hot columns: W[p, kt, 480+g] = (g == kt).  DMA into a
            # contiguous staging tile (128x2KB descriptors), then one DVE
            # copy scatters it into W's strided column block -- a direct
            # strided DMA would be 4096x64B descriptors and gate the PE.
            ohc = consts.tile([128, G, G], BF16)
            nc.sync.dma_start(out=ohc, in_=oh_d[:, :, :])
            nc.vector.tensor_copy(W[:, :, 480:512], ohc)
            nc.vector.memset(xsp, 0.0)

            pb = ctx.enter_context(
                tc.tile_pool(name="pb", bufs=1, space="PSUM")
            )

            qw_load_range(0, 2)   # pair 0 alone: unblocks the first dequant
            qw_load_range(2, 8)   # right behind on the same ring (FIFO)
            # preload the activation table used by scale=-1 copies so the
            # 1.3us lazy ACT_TABLE_LOAD doesn't land on a phase boundary
            nc.scalar.activation(
                xsp[:, 0, 0:1], xsp[:, 0, 0:1],
                mybir.ActivationFunctionType.Copy, scale=-1.0,
            )
            nc.scalar.dma_start(out=qz_sb, in_=qz_d[:, :])
            for p in range(3):
                nc.scalar.dma_start(out=sps[p], in_=sp_d[p][0])
            with tc.tile_wait_until(0.008):
                for c in range(1, KT // QCH):
                    qw_load_range(c * QCH, (c + 1) * QCH)
            zb_prep()

            def drain_m(ph, m, ps):
                plo, phi, _, _ = PH[ph]
                width = phi - plo
                # zero-point correction for this phase's columns; lhsT is
                # the quadrant-aligned 32-row slice of the grouped xsT
                if not _NO_CORR:
                    nc.tensor.matmul(
                        ps[:, 0:width],
                        lhsT=xsT[0:G, m, :],
                        rhs=Bs[ph],
                        start=False, stop=True,
                        skip_group_check=True,
                    )
                osb = opool.tile([128, 512], BF16, name="osb", tag="osb")
                nc.vector.tensor_copy(osb[:, 0:width], ps[:, 0:width])
                nc.sync.dma_start(
                    out=out_d[m * 128:(m + 1) * 128, plo:phi],
                    in_=osb[:, 0:width],
                )

            def mk_ps(ph, m):
                return pb.tile([128, 512], F32, name=f"ps_{ph}_{m}",
                               tag=f"ps{m}", bufs=1)

            # ---- phase 0: kt-outer / m-inner (matches dequant+x pace) ----
            ps0 = [mk_ps(0, m) for m in range(MT)]
            for kt in range(KT):
                if kt % 2 == 0:
                    dequant_pair(0, kt // 2)
                if kt % XCH == 0:
                    x_load(kt // XCH)
                for m in range(MT):
                    nc.tensor.matmul(
                        ps0[m][:, 0:512],
                        lhsT=xT[:, kt, m * 128:(m + 1) * 128],
                        rhs=W[:, kt, 0:512],
                        start=(kt == 0), stop=False,
                        skip_group_check=True,
                    )
            # phase 1/2 dequant prefetch (issued under timeline pins; runs
            # during phase 0 on DVE + sync-ring slack)
            for ph in (1, 2):
                with tc.tile_wait_until(PH_PIN[ph]):
                    for j in range(KT // 2):
                        dequant_pair(ph, j)
            # phase-0 drains, fully interleaved per m-tile: m0's chain
            # gates phase 1's start; later m-tiles have a kt-sweep of slack
            for m in range(MT):
                nc.scalar.activation(
                    xsp[:, m, 0:G], ps0[m][:, 480:512],
                    mybir.ActivationFunctionType.Copy, scale=-1.0,
                )
                nc.scalar.dma_start(
                    out=xsT[:, m, :], in_=xsp[:, m, :], transpose=True
                )
                drain_m(0, m, ps0[m])

            # ---- phases 1, 2: m-outer / kt-inner (W fully resident) ----
            # each phase boundary gates on a single m-tile's drain and the
            # drains spread one per kt-sweep instead of piling up
            for ph in (1, 2):
                wlo, wwidth = PH[ph][2], PH[ph][3]
                for m in range(MT):
                    ps = mk_ps(ph, m)
                    for kt in range(KT):
                        nc.tensor.matmul(
                            ps[:, 0:wwidth],
                            lhsT=xT[:, kt, m * 128:(m + 1) * 128],
                            rhs=W[:, kt, wlo:wlo + wwidth],
                            start=(kt == 0), stop=False,
                            skip_group_check=True,
                        )
                    drain_m(ph, m, ps)

    return nc


def _get_nc():
    global _cached_nc
    if _cached_nc is None:
        _cached_nc = _build()
    return _cached_nc


def kernel(x, qweight, scales, qzeros):
    global LAST_EXEC_NS, LAST_TRACE

    x = np.asarray(x, dtype=np.float32)
    # partition-major x: [128, KT, M]
    x_t = np.ascontiguousarray(
        x.T.astype(ml_dtypes.bfloat16).reshape(KT, 128, M).transpose(1, 0, 2)
    )
    qweight = np.asarray(qweight, dtype=np.int32)
    scales = np.asarray(scales, dtype=np.float32)
    qzeros = np.asarray(qzeros, dtype=np.int32)
    oh = np.ascontiguousarray(
        np.broadcast_to(np.eye(G, dtype=ml_dtypes.bfloat16)[None], (128, G, G))
    )

    in_maps = []
    for c in range(NCORES):
        # partition-major qweight: qw_pm[p, a, :] = qweight[a*128 + p, cols]
        qw_c = qweight[:, c * NP:(c + 1) * NP]
        qw_pm = np.ascontiguousarray(
            qw_c.reshape(KT, 128, NP).transpose(1, 0, 2)
        )
        qz_c = np.ascontiguousarray(qzeros[:, c * NP:(c + 1) * NP])
        s_c = scales[:, c * N_LOC:(c + 1) * N_LOC]
        # pair-block permutation: dest[g, 344*t + 2*cc + r] = s[g, 8*cc + 2*t + r]
        s_perm = np.ascontiguousarray(
            s_c.reshape(G, NP, 4, 2).transpose(0, 2, 1, 3).reshape(G, N_LOC)
        ).astype(ml_dtypes.bfloat16)
        im = {"xt": x_t, "qw": qw_pm, "qz": qz_c, "oh": oh}
        for p in range(3):
            plo, phi = PH[p][0], PH[p][1]
            im[f"sp{p}"] = np.ascontiguousarray(
                np.broadcast_to(s_perm[None, :, plo:phi],
                                (SREP, G, phi - plo))
            )
        in_maps.append(im)

    trace = os.environ.get("AWQ_KERNEL_TRACE", "0") == "1"
    if trace:
        _install_ntff_shim()

    nc = _get_nc()
    res = run_bass_kernel_spmd(
        nc, in_maps, core_ids=list(range(NCORES)), trace=trace
    )
    LAST_EXEC_NS = res.exec_time_ns
    if res.instructions_and_trace is not None:
        LAST_TRACE = res.instructions_and_trace[1]

    # undo the pair-block column permutation on the host (pure reindex)
    return np.concatenate(
        [np.asarray(res.results[i]["out"]).astype(np.float32)[:, _INV]
         for i in range(NCORES)],
        axis=1,
    )


# revision 34
# speedup vs baseline: 1.1926x; 1.1926x over previous
"""AWQ int4 GEMM (M=1024, K=4096, N=11008, group_size=128) on 8 TRN2 NeuronCores.

Column-parallel tensor sharding (vLLM-style): qweight/qzeros/scales split
along N across the 8 cores, activations replicated, outputs concatenated.
Each core is fully independent (no collectives).

v3 structure — three N-phases so the PE never waits on dequant DMA:
  The dequantized weight tile W [128, KT, 1408] stays fully resident in
  SBUF, and the PE sweeps it in three column passes, each a plain
  kt(outer) x m(inner) accumulation into 8 PSUM banks (one per m-tile):
    phase 0: cols    0: 512  = perm-cols 0:480  ++ 32 one-hot columns
    phase 1: cols  512:1024  = perm-cols 480:992
    phase 2: cols 1024:1408  = perm-cols 992:1376
  The scale broadcast (128x-replicated rows, 11.3 MB) is split into
  per-phase column slices stored phase-sliced in DRAM so every transfer is
  single-segment; dequant for phase p+1 prefetches while the PE consumes
  phase p (prefetch issue pinned on the modeled timeline so its DMA-queue
  credit slots never land between the phase-0 drain chain's transfers —
  the HW queue-completion semaphores are ordered by issue position, and a
  mis-ordered prefetch flood head-of-line-blocks the drains).  qweight is
  loaded once (22KB/partition, big-descriptor chunks, a small lead chunk
  first) and stays resident; x is partition-major in DRAM so its loads
  are 8KB-descriptor chunks.  Engine roles are split so prefetch can
  never block drains: sync ring = input streaming (x, scale slices, oh)
  + output stores, scalar(Act) ring = qweight + xsum copies + XBAR
  transposes, DVE = dequant + PSUM->SBUF output copies.

  Phase 0 runs kt-outer/m-inner (matches the dequant arrival pace);
  phases 1-2 run m-outer/kt-inner (W is resident by then), so each phase
  boundary gates on a single m-tile's drain and the other seven drains
  spread across the following kt-sweeps.

  The one-hot columns compute xsum on the PE for free: column 480+g of
  k-tile kt's rhs is ones iff g==kt, so after phase 0 psum cols 480:512
  hold xsum[m-row, g].  Per m-tile that block is copied to bf16 with
  scale=-1 (free negation on the Act engine), transposed by the DMA XBAR,
  and used as the stationary operand of the per-phase zero-point
  correction matmul: out += (-xsum_g) @ (z_g * s_g).

  Outputs are written per (m-tile, phase) straight from PSUM->SBUF->DRAM
  in pair-block-permuted column order; the host undoes the permutation
  (pure reindex, no FLOPs), which removes the on-device unpermute copies
  and the full-width output staging tile.

  Measured: ~196.8us HW exec (baseline v1: ~226us), rel err 4.7e-3.
  PE busy ~166us of which ~153us is the minimal bf16 matmul stream
  (K*M*(N_loc+32)/128^2 cycles + rank-32 corrections at 2.4GHz).
"""

import os
import sys
import types

sys.path.insert(0, "/opt/trn_rl_repo")

import numpy as np
import ml_dtypes

import concourse.bass as bass
import concourse.bass_isa as bass_isa
import concourse.mybir as mybir
import concourse.tile as tile
import bass_rust as _br
from concourse.vector_clock import ScopedClock
from concourse.bass_utils import run_bass_kernel_spmd


# ---------------------------------------------------------------------------
# Walrus workaround: this toolchain rejects >1 sem wait per instruction
# (2 for InstEventSemaphore).  Tile's sem assigner can emit more; split the
# excess onto no-ops placed immediately before on the same engine.
# ---------------------------------------------------------------------------
_orig_lower = tile.TileContext._lower_ordered_insts
_wsplit_counter = [0]


def _split_waits_in_place(nc, insts):
    new_list = []
    for inst in insts:
        si = inst.sync_info
        cap = 2 if isinstance(inst, mybir.InstEventSemaphore) else 1
        if si is not None and len(si.on_wait) > cap:
            waits = list(si.on_wait)
            extra, keep = waits[:-cap], waits[-cap:]
            for w in extra:
                _wsplit_counter[0] += 1
                nop = mybir.InstNoOp(
                    name=f"wsplit-{_wsplit_counter[0]}",
                    engine=inst.engine,
                    sync_info=mybir.SyncInfo(on_wait=[w], on_update=[]),
                    bass_nofuse=True,
                )
                nc.register_instruction(nop)
                new_list.append(nop)
            inst.sync_info = mybir.SyncInfo(on_wait=keep, on_update=list(si.on_update))
        new_list.append(inst)
    insts[:] = new_list


def _dedup_ldweights_in_place(nc, insts):
    """Drop InstLdweights identical to the still-loaded stationary operand."""
    last_sig = None
    out = []
    for inst in insts:
        nm = inst.__class__.__name__
        if nm == "InstLdweights":
            sig = repr(inst.ins[0])
            if sig == last_sig:
                si = inst.sync_info
                if si is not None and (si.on_wait or si.on_update):
                    _wsplit_counter[0] += 1
                    nop = mybir.InstNoOp(
                        name=f"ldwkill-{_wsplit_counter[0]}",
                        engine=inst.engine,
                        sync_info=mybir.SyncInfo(
                            on_wait=list(si.on_wait),
                            on_update=list(si.on_update),
                        ),
                        bass_nofuse=True,
                    )
                    nc.register_instruction(nop)
                    out.append(nop)
                continue
            last_sig = sig
        elif nm != "InstMatmult" and inst.engine == mybir.EngineType.PE:
            last_sig = None
        out.append(inst)
    insts[:] = out


def _patched_lower(self, ordered):
    for insts in ordered.values():
        _dedup_ldweights_in_place(self.nc, insts)
        _split_waits_in_place(self.nc, insts)
    return _orig_lower(self, ordered)


def _patched_drain_and_barrier(self, tick_clock, wait_clock):
    nc = self.nc
    drain_inst = nc.sync.drain()
    wait_clock.add_sem_waits(
        drain_inst.ins, ScopedClock({None: tick_clock.global_clock})
    )
    si = drain_inst.ins.sync_info
    if si is not None and len(si.on_wait) > 1:
        waits = list(si.on_wait)
        drain_inst.ins.sync_info = _br.SyncInfo(
            on_wait=[waits[0]], on_update=list(si.on_update)
        )
        for w in waits[1:]:
            extra = nc.sync.drain()
            extra.ins.sync_info = _br.SyncInfo(on_wait=[w], on_update=[])
    nc.all_engine_barrier()
    assert self.sems is not None
    popped = nc._tile_sem_poison_stack.pop()
    assert popped is self._sem_poison
    nc.clear_and_free_semaphores(list(self.sems.allocated().values()))
    nc.all_engine_barrier()


tile.TileContext._lower_ordered_insts = _patched_lower
tile.TileContext._drain_and_barrier = _patched_drain_and_barrier

# ---------------------------------------------------------------------------
# NTFF profiling hook shim (only used when tracing is requested).
# ---------------------------------------------------------------------------
def _install_ntff_shim():
    if "antenv.axon_hooks" in sys.modules:
        return
    try:
        from trn_agent_boot.trn_boot import _ntff_profile_via_ctypes

        hook = _ntff_profile_via_ctypes("/opt/axon/libaxon_pjrt.so")
    except Exception:
        hook = None
    m = types.ModuleType("antenv.axon_hooks")
    m.get_axon_ntff_profile_hook = lambda: hook
    m.set_axon_ntff_profile_hook = lambda h: None
    import antenv  # noqa: F401

    sys.modules["antenv.axon_hooks"] = m


# ---------------------------------------------------------------------------
# Problem shape (hardcoded per contract)
# ---------------------------------------------------------------------------
M, K, N_TOTAL = 1024, 4096, 11008
NCORES = 8
N_LOC = N_TOTAL // NCORES  # 1376 unpacked columns per core
NP = N_LOC // 8            # 172 packed int32 columns per core
G = 32                     # scale/zero groups (group_size 128 == k-tile)
KT = K // 128              # 32 k-tiles
MT = M // 128              # 8 m-tiles
NB = 4                     # pair-blocks per core (one per unpack shift)
BW = N_LOC // NB           # 344 perm columns per pair-block

PAIR_SHIFTS = (0, 4, 8, 12)
SREP = 128  # scale-row replication factor in DRAM
PAIR_MASK = 0x000F000F

F32 = mybir.dt.float32
BF16 = mybir.dt.bfloat16
I32 = mybir.dt.int32
I16 = mybir.dt.int16

# Column phases: (perm_lo, perm_hi, W col offset, matmul width)
NEXT = N_LOC + 32          # 1408 = W row width
PH = (
    (0, 480, 0, 512),      # + one-hot cols 480:512 of W
    (480, 992, 512, 512),
    (992, 1376, 1024, 384),
)
# packed-column pieces (t, c0, c1) per phase: perm cols [344t+2c0, 344t+2c1)
PH_PIECES = (
    ((0, 0, 172), (1, 0, 68)),
    ((1, 68, 172), (2, 0, 152)),
    ((2, 152, 172), (3, 0, 172)),
)
# modeled-timeline pins (ms) for phase 1/2 dequant prefetch
PH_PIN = (None, 0.015, 0.085)
XCH = 4                    # k-tiles per x load chunk
QCH = 8                    # k-tiles per qweight load chunk

# inverse permutation: orig col o -> perm col
_INV = np.array(
    [344 * ((o % 8) // 2) + 2 * (o // 8) + (o % 2) for o in range(N_LOC)],
    dtype=np.int64,
)

# debug switches
_NO_CORR = os.environ.get("AWQ_NO_CORR", "0") == "1"   # skip zero-point corr

LAST_EXEC_NS = None
LAST_TRACE = None

_cached_nc = None


def _build():
    nc = bass.Bass()
    # x partition-major: [128, KT, M], partition p holds rows {kt*128+p}
    xt_d = nc.declare_dram_parameter("xt", [128, KT, M], BF16, isOutput=False)
    qw_d = nc.declare_dram_parameter("qw", [128, KT, NP], I32, isOutput=False)
    # per-phase scale slices, replica-major [SREP, G, w] (perm order)
    sp_d = [
        nc.declare_dram_parameter(f"sp{p}", [SREP, G, PH[p][1] - PH[p][0]],
                                  BF16, isOutput=False)
        for p in range(3)
    ]
    qz_d = nc.declare_dram_parameter("qz", [G, NP], I32, isOutput=False)
    oh_d = nc.declare_dram_parameter("oh", [128, G, G], BF16, isOutput=False)
    # output in PERM column order; host undoes the permutation
    out_d = nc.declare_dram_parameter("out", [M, N_LOC], BF16, isOutput=True)

    AND = mybir.AluOpType.bitwise_and
    LSR = mybir.AluOpType.logical_shift_right
    MUL = mybir.AluOpType.mult

    with tile.TileContext(nc) as tc:
        from contextlib import ExitStack

        with ExitStack() as ctx:
            big = ctx.enter_context(tc.tile_pool(name="big", bufs=1))
            xT = big.tile([128, KT, M], BF16)        # x (k on partitions)
            W = big.tile([128, KT, NEXT], BF16)      # dequant + one-hot cols
            qw_sb = big.tile([128, KT, NP], I32)     # packed qweight, resident

            consts = ctx.enter_context(tc.tile_pool(name="consts", bufs=1))
            qz_sb = consts.tile([G, NP], I32)
            znib = consts.tile([G, NB, NP], I32)
            sps = [consts.tile([G, PH[p][1] - PH[p][0]], BF16, name=f"sps{p}")
                   for p in range(3)]
            Bs = [consts.tile([G, PH[p][1] - PH[p][0]], BF16, name=f"Bs{p}")
                  for p in range(3)]                 # +(z*s) per phase slice
            xsp = consts.tile([128, MT, 128], BF16)  # -xsum staging (padded)
            xsT = consts.tile([128, MT, 128], BF16)  # transposed -xsum per m

            wprep = ctx.enter_context(tc.tile_pool(name="wprep", bufs=2))
            opool = ctx.enter_context(tc.tile_pool(name="oout", bufs=3))

            # priming: first qweight chunk + first scale slice lead the
            # scalar ring; one-hot identity + first x chunk lead sync.
            def qw_load_range(k0, k1):
                nc.scalar.dma_start(
                    out=qw_sb[:, k0:k1, :],
                    in_=qw_d[:, k0:k1, :],
                )

            def x_load(c):
                nc.sync.dma_start(
                    out=xT[:, c * XCH:(c + 1) * XCH, :],
                    in_=xt_d[:, c * XCH:(c + 1) * XCH, :],
                )

            def dequant_pair(ph, j, s2_eng=None):
                plo, phi, wlo, _ = PH[ph]
                w = phi - plo
                pieces = PH_PIECES[ph]
                pw = sum(c1 - c0 for _, c0, c1 in pieces)
                kt = 2 * j
                nib = wprep.tile([128, 2, 256], I32, name="nib", tag="nib",
                                 bufs=2)
                off = 0
                for t, c0, c1 in pieces:
                    nc.vector.tensor_scalar(
                        out=nib[:, :, off:off + (c1 - c0)],
                        in0=qw_sb[:, kt:kt + 2, c0:c1],
                        scalar1=PAIR_SHIFTS[t], scalar2=PAIR_MASK,
                        op0=LSR, op1=AND,
                    )
                    off += c1 - c0
                # scale rows kt, kt+1 of this phase's slice: contiguous
                # single-segment 2*w*2B run per partition (own replica)
                s2 = wprep.tile([128, 2, 512], BF16, name="s2", tag="sbc",
                                bufs=6)
                rep = sp_d[ph][0]  # [G, w] view of replica 0
                (s2_eng or nc.sync).dma_start(
                    out=s2[:, :, 0:w],
                    in_=bass.AP(
                        tensor=rep.tensor,
                        offset=rep.offset + kt * w,
                        ap=[[G * w, 128], [w, 2], [1, w]],
                    ),
                )
                nib16 = nib.bitcast(I16)  # [128, 2, 512]
                nc.vector.tensor_tensor(
                    out=W[:, kt:kt + 2, wlo:wlo + 2 * pw],
                    in0=nib16[:, :, 0:2 * pw],
                    in1=s2[:, :, 0:w],
                    op=MUL,
                )

            def zb_prep():
                for t in range(NB):
                    nc.vector.tensor_scalar(
                        out=znib[:, t, :], in0=qz_sb,
                        scalar1=PAIR_SHIFTS[t], scalar2=PAIR_MASK,
                        op0=LSR, op1=AND,
                    )
                z16 = znib.bitcast(I16).rearrange("p a b -> p (a b)")
                for p in range(3):
                    plo, phi = PH[p][0], PH[p][1]
                    nc.vector.tensor_tensor(
                        out=Bs[p], in0=z16[:, plo:phi], in1=sps[p], op=MUL
                    )

            # one-hot columns: W[p, kt, 480+g] = (g == kt).  DMA into a
            # contiguous staging tile (128x2KB descriptors), then one DVE
            # copy scatters it into W's strided column block -- a direct
            # strided DMA would be 4096x64B descriptors and gate the PE.
            ohc = consts.tile([128, G, G], BF16)
            nc.sync.dma_start(out=ohc, in_=oh_d[:, :, :])
            nc.vector.tensor_copy(W[:, :, 480:512], ohc)
            nc.vector.memset(xsp, 0.0)

            pb = ctx.enter_context(
                tc.tile_pool(name="pb", bufs=1, space="PSUM")
            )

            qw_load_range(0, 2)   # pair 0 alone: unblocks the first dequant
            qw_load_range(2, 8)   # right behind on the same ring (FIFO)
            # preload the activation table used by scale=-1 copies so the
            # 1.3us lazy ACT_TABLE_LOAD doesn't land on a phase boundary
            nc.scalar.activation(
                xsp[:, 0, 0:1], xsp[:, 0, 0:1],
                mybir.ActivationFunctionType.Copy, scale=-1.0,
            )
            nc.scalar.dma_start(out=qz_sb, in_=qz_d[:, :])
            for p in range(3):
                nc.scalar.dma_start(out=sps[p], in_=sp_d[p][0])
            with tc.tile_wait_until(0.008):
                for c in range(1, KT // QCH):
                    qw_load_range(c * QCH, (c + 1) * QCH)
            zb_prep()

            def drain_m(ph, m, ps):
                plo, phi, _, _ = PH[ph]
                width = phi - plo
                # zero-point correction for this phase's columns; lhsT is
                # the quadrant-aligned 32-row slice of the grouped xsT
                if not _NO_CORR:
                    nc.tensor.matmul(
                        ps[:, 0:width],
                        lhsT=xsT[0:G, m, :],
                        rhs=Bs[ph],
                        start=False, stop=True,
                        skip_group_check=True,
                    )
                osb = opool.tile([128, 512], BF16, name="osb", tag="osb")
                nc.vector.tensor_copy(osb[:, 0:width], ps[:, 0:width])
                nc.sync.dma_start(
                    out=out_d[m * 128:(m + 1) * 128, plo:phi],
                    in_=osb[:, 0:width],
                )

            def mk_ps(ph, m):
                return pb.tile([128, 512], F32, name=f"ps_{ph}_{m}",
                               tag=f"ps{m}", bufs=1)

            # ---- phase 0: kt-outer / m-inner (matches dequant+x pace) ----
            ps0 = [mk_ps(0, m) for m in range(MT)]
            for kt in range(KT):
                if kt % 2 == 0:
                    dequant_pair(0, kt // 2)
                if kt % XCH == 0:
                    x_load(kt // XCH)
                for m in range(MT):
                    nc.tensor.matmul(
                        ps0[m][:, 0:512],
                        lhsT=xT[:, kt, m * 128:(m + 1) * 128],
                        rhs=W[:, kt, 0:512],
                        start=(kt == 0), stop=False,
                        skip_group_check=True,
                    )
            # phase 1/2 dequant prefetch (issued under timeline pins; runs
            # during phase 0 on DVE + sync-ring slack)
            for ph in (1, 2):
                with tc.tile_wait_until(PH_PIN[ph]):
                    for j in range(KT // 2):
                        dequant_pair(ph, j)
            # phase-0 drains, fully interleaved per m-tile: m0's chain
            # gates phase 1's start; later m-tiles have a kt-sweep of slack
            for m in range(MT):
                nc.scalar.activation(
                    xsp[:, m, 0:G], ps0[m][:, 480:512],
                    mybir.ActivationFunctionType.Copy, scale=-1.0,
                )
                nc.scalar.dma_start(
                    out=xsT[:, m, :], in_=xsp[:, m, :], transpose=True
                )
                drain_m(0, m, ps0[m])

            # ---- phases 1, 2: m-outer / kt-inner (W fully resident) ----
            # each phase boundary gates on a single m-tile's drain and the
            # drains spread one per kt-sweep instead of piling up
            for ph in (1, 2):
                wlo, wwidth = PH[ph][2], PH[ph][3]
                for m in range(MT):
                    ps = mk_ps(ph, m)
                    for kt in range(KT):
                        nc.tensor.matmul(
                            ps[:, 0:wwidth],
                            lhsT=xT[:, kt, m * 128:(m + 1) * 128],
                            rhs=W[:, kt, wlo:wlo + wwidth],
                            start=(kt == 0), stop=False,
                            skip_group_check=True,
                        )
                    drain_m(ph, m, ps)

    return nc


def _get_nc():
    global _cached_nc
    if _cached_nc is None:
        _cached_nc = _build()
    return _cached_nc


def kernel(x, qweight, scales, qzeros):
    global LAST_EXEC_NS, LAST_TRACE

    x = np.asarray(x, dtype=np.float32)
    # partition-major x: [128, KT, M]
    x_t = np.ascontiguousarray(
        x.T.astype(ml_dtypes.bfloat16).reshape(KT, 128, M).transpose(1, 0, 2)
    )
    qweight = np.asarray(qweight, dtype=np.int32)
    scales = np.asarray(scales, dtype=np.float32)
    qzeros = np.asarray(qzeros, dtype=np.int32)
    oh = np.ascontiguousarray(
        np.broadcast_to(np.eye(G, dtype=ml_dtypes.bfloat16)[None], (128, G, G))
    )

    in_maps = []
    for c in range(NCORES):
        # partition-major qweight: qw_pm[p, a, :] = qweight[a*128 + p, cols]
        qw_c = qweight[:, c * NP:(c + 1) * NP]
        qw_pm = np.ascontiguousarray(
            qw_c.reshape(KT, 128, NP).transpose(1, 0, 2)
        )
        qz_c = np.ascontiguousarray(qzeros[:, c * NP:(c + 1) * NP])
        s_c = scales[:, c * N_LOC:(c + 1) * N_LOC]
        # pair-block permutation: dest[g, 344*t + 2*cc + r] = s[g, 8*cc + 2*t + r]
        s_perm = np.ascontiguousarray(
            s_c.reshape(G, NP, 4, 2).transpose(0, 2, 1, 3).reshape(G, N_LOC)
        ).astype(ml_dtypes.bfloat16)
        im = {"xt": x_t, "qw": qw_pm, "qz": qz_c, "oh": oh}
        for p in range(3):
            plo, phi = PH[p][0], PH[p][1]
            im[f"sp{p}"] = np.ascontiguousarray(
                np.broadcast_to(s_perm[None, :, plo:phi],
                                (SREP, G, phi - plo))
            )
        in_maps.append(im)

    trace = os.environ.get("AWQ_KERNEL_TRACE", "0") == "1"
    if trace:
        _install_ntff_shim()

    nc = _get_nc()
    res = run_bass_kernel_spmd(
        nc, in_maps, core_ids=list(range(NCORES)), trace=trace
    )
    LAST_EXEC_NS = res.exec_time_ns
    if res.instructions_and_trace is not None:
        LAST_TRACE = res.instructions_and_trace[1]

    # undo the pair-block column permutation on the host (pure reindex)
    return np.concatenate(
        [np.asarray(res.results[i]["out"]).astype(np.float32)[:, _INV]
         for i in range(NCORES)],
        axis=1,
    )


# revision 35
# speedup vs baseline: 1.2000x; 1.0062x over previous
"""AWQ int4 GEMM (M=1024, K=4096, N=11008, group_size=128) on 8 TRN2 NeuronCores.

Column-parallel tensor sharding (vLLM-style): qweight/qzeros/scales split
along N across the 8 cores, activations replicated, outputs concatenated.
Each core is fully independent (no collectives).

v3 structure — three N-phases so the PE never waits on dequant DMA:
  The dequantized weight tile W [128, KT, 1408] stays fully resident in
  SBUF, and the PE sweeps it in three column passes, each a plain
  kt(outer) x m(inner) accumulation into 8 PSUM banks (one per m-tile):
    phase 0: cols    0: 512  = perm-cols 0:480  ++ 32 one-hot columns
    phase 1: cols  512:1024  = perm-cols 480:992
    phase 2: cols 1024:1408  = perm-cols 992:1376
  The scale broadcast (128x-replicated rows, 11.3 MB) is split into
  per-phase column slices stored phase-sliced in DRAM so every transfer is
  single-segment; dequant for phase p+1 prefetches while the PE consumes
  phase p (prefetch issue pinned on the modeled timeline so its DMA-queue
  credit slots never land between the phase-0 drain chain's transfers —
  the HW queue-completion semaphores are ordered by issue position, and a
  mis-ordered prefetch flood head-of-line-blocks the drains).  qweight is
  loaded once (22KB/partition, big-descriptor chunks, a small lead chunk
  first) and stays resident; x is partition-major in DRAM so its loads
  are 8KB-descriptor chunks.  Engine roles are split so prefetch can
  never block drains: sync ring = input streaming (x, scale slices, oh)
  + output stores, scalar(Act) ring = qweight + xsum copies + XBAR
  transposes, DVE = dequant + PSUM->SBUF output copies.

  Phase 0 runs kt-outer/m-inner (matches the dequant arrival pace);
  phases 1-2 run m-outer/kt-inner (W is resident by then), so each phase
  boundary gates on a single m-tile's drain and the other seven drains
  spread across the following kt-sweeps.

  The one-hot columns compute xsum on the PE for free: column 480+g of
  k-tile kt's rhs is ones iff g==kt, so after phase 0 psum cols 480:512
  hold xsum[m-row, g].  Per m-tile that block is copied to bf16 with
  scale=-1 (free negation on the Act engine), transposed by the DMA XBAR,
  and used as the stationary operand of the per-phase zero-point
  correction matmul: out += (-xsum_g) @ (z_g * s_g).

  Outputs are written per (m-tile, phase) straight from PSUM->SBUF->DRAM
  in pair-block-permuted column order; the host undoes the permutation
  (pure reindex, no FLOPs), which removes the on-device unpermute copies
  and the full-width output staging tile.

  Measured: ~196.8us HW exec (baseline v1: ~226us), rel err 4.7e-3.
  PE busy ~166us of which ~153us is the minimal bf16 matmul stream
  (K*M*(N_loc+32)/128^2 cycles + rank-32 corrections at 2.4GHz).
"""

import os
import sys
import types

sys.path.insert(0, "/opt/trn_rl_repo")

import numpy as np
import ml_dtypes

import concourse.bass as bass
import concourse.bass_isa as bass_isa
import concourse.mybir as mybir
import concourse.tile as tile
import bass_rust as _br
from concourse.vector_clock import ScopedClock
from concourse.bass_utils import run_bass_kernel_spmd


# ---------------------------------------------------------------------------
# Walrus workaround: this toolchain rejects >1 sem wait per instruction
# (2 for InstEventSemaphore).  Tile's sem assigner can emit more; split the
# excess onto no-ops placed immediately before on the same engine.
# ---------------------------------------------------------------------------
_orig_lower = tile.TileContext._lower_ordered_insts
_wsplit_counter = [0]


def _split_waits_in_place(nc, insts):
    new_list = []
    for inst in insts:
        si = inst.sync_info
        cap = 2 if isinstance(inst, mybir.InstEventSemaphore) else 1
        if si is not None and len(si.on_wait) > cap:
            waits = list(si.on_wait)
            extra, keep = waits[:-cap], waits[-cap:]
            for w in extra:
                _wsplit_counter[0] += 1
                nop = mybir.InstNoOp(
                    name=f"wsplit-{_wsplit_counter[0]}",
                    engine=inst.engine,
                    sync_info=mybir.SyncInfo(on_wait=[w], on_update=[]),
                    bass_nofuse=True,
                )
                nc.register_instruction(nop)
                new_list.append(nop)
            inst.sync_info = mybir.SyncInfo(on_wait=keep, on_update=list(si.on_update))
        new_list.append(inst)
    insts[:] = new_list


def _dedup_ldweights_in_place(nc, insts):
    """Drop InstLdweights identical to the still-loaded stationary operand."""
    last_sig = None
    out = []
    for inst in insts:
        nm = inst.__class__.__name__
        if nm == "InstLdweights":
            sig = repr(inst.ins[0])
            if sig == last_sig:
                si = inst.sync_info
                if si is not None and (si.on_wait or si.on_update):
                    _wsplit_counter[0] += 1
                    nop = mybir.InstNoOp(
                        name=f"ldwkill-{_wsplit_counter[0]}",
                        engine=inst.engine,
                        sync_info=mybir.SyncInfo(
                            on_wait=list(si.on_wait),
                            on_update=list(si.on_update),
                        ),
                        bass_nofuse=True,
                    )
                    nc.register_instruction(nop)
                    out.append(nop)
                continue
            last_sig = sig
        elif nm != "InstMatmult" and inst.engine == mybir.EngineType.PE:
            last_sig = None
        out.append(inst)
    insts[:] = out


def _patched_lower(self, ordered):
    for insts in ordered.values():
        _dedup_ldweights_in_place(self.nc, insts)
        _split_waits_in_place(self.nc, insts)
    return _orig_lower(self, ordered)


def _patched_drain_and_barrier(self, tick_clock, wait_clock):
    nc = self.nc
    drain_inst = nc.sync.drain()
    wait_clock.add_sem_waits(
        drain_inst.ins, ScopedClock({None: tick_clock.global_clock})
    )
    si = drain_inst.ins.sync_info
    if si is not None and len(si.on_wait) > 1:
        waits = list(si.on_wait)
        drain_inst.ins.sync_info = _br.SyncInfo(
            on_wait=[waits[0]], on_update=list(si.on_update)
        )
        for w in waits[1:]:
            extra = nc.sync.drain()
            extra.ins.sync_info = _br.SyncInfo(on_wait=[w], on_update=[])
    nc.all_engine_barrier()
    assert self.sems is not None
    popped = nc._tile_sem_poison_stack.pop()
    assert popped is self._sem_poison
    nc.clear_and_free_semaphores(list(self.sems.allocated().values()))
    nc.all_engine_barrier()


tile.TileContext._lower_ordered_insts = _patched_lower
tile.TileContext._drain_and_barrier = _patched_drain_and_barrier

# ---------------------------------------------------------------------------
# NTFF profiling hook shim (only used when tracing is requested).
# ---------------------------------------------------------------------------
def _install_ntff_shim():
    if "antenv.axon_hooks" in sys.modules:
        return
    try:
        from trn_agent_boot.trn_boot import _ntff_profile_via_ctypes

        hook = _ntff_profile_via_ctypes("/opt/axon/libaxon_pjrt.so")
    except Exception:
        hook = None
    m = types.ModuleType("antenv.axon_hooks")
    m.get_axon_ntff_profile_hook = lambda: hook
    m.set_axon_ntff_profile_hook = lambda h: None
    import antenv  # noqa: F401

    sys.modules["antenv.axon_hooks"] = m


# ---------------------------------------------------------------------------
# Problem shape (hardcoded per contract)
# ---------------------------------------------------------------------------
M, K, N_TOTAL = 1024, 4096, 11008
NCORES = 8
N_LOC = N_TOTAL // NCORES  # 1376 unpacked columns per core
NP = N_LOC // 8            # 172 packed int32 columns per core
G = 32                     # scale/zero groups (group_size 128 == k-tile)
KT = K // 128              # 32 k-tiles
MT = M // 128              # 8 m-tiles
NB = 4                     # pair-blocks per core (one per unpack shift)
BW = N_LOC // NB           # 344 perm columns per pair-block

PAIR_SHIFTS = (0, 4, 8, 12)
SREP = 128  # scale-row replication factor in DRAM
PAIR_MASK = 0x000F000F

F32 = mybir.dt.float32
BF16 = mybir.dt.bfloat16
I32 = mybir.dt.int32
I16 = mybir.dt.int16

# Column phases: (perm_lo, perm_hi, W col offset, matmul width)
NEXT = N_LOC + 32          # 1408 = W row width
PH = (
    (0, 480, 0, 512),      # + one-hot cols 480:512 of W
    (480, 992, 512, 512),
    (992, 1376, 1024, 384),
)
# packed-column pieces (t, c0, c1) per phase: perm cols [344t+2c0, 344t+2c1)
PH_PIECES = (
    ((0, 0, 172), (1, 0, 68)),
    ((1, 68, 172), (2, 0, 152)),
    ((2, 152, 172), (3, 0, 172)),
)
# modeled-timeline pins (ms) for phase 1/2 dequant prefetch
PH_PIN = (None, 0.015, 0.085)
XCH = 4                    # k-tiles per x load chunk
QCH = 8                    # k-tiles per qweight load chunk

# inverse permutation: orig col o -> perm col
_INV = np.array(
    [344 * ((o % 8) // 2) + 2 * (o // 8) + (o % 2) for o in range(N_LOC)],
    dtype=np.int64,
)

# debug switches
_NO_CORR = os.environ.get("AWQ_NO_CORR", "0") == "1"   # skip zero-point corr

LAST_EXEC_NS = None
LAST_TRACE = None

_cached_nc = None


def _build():
    nc = bass.Bass()
    # x partition-major: [128, KT, M], partition p holds rows {kt*128+p}
    xt_d = nc.declare_dram_parameter("xt", [128, KT, M], BF16, isOutput=False)
    qw_d = nc.declare_dram_parameter("qw", [128, KT, NP], I32, isOutput=False)
    # per-phase scale slices, replica-major [SREP, G, w] (perm order)
    sp_d = [
        nc.declare_dram_parameter(f"sp{p}", [SREP, G, PH[p][1] - PH[p][0]],
                                  BF16, isOutput=False)
        for p in range(3)
    ]
    qz_d = nc.declare_dram_parameter("qz", [G, NP], I32, isOutput=False)
    oh_d = nc.declare_dram_parameter("oh", [128, G, G], BF16, isOutput=False)
    # output in PERM column order; host undoes the permutation
    out_d = nc.declare_dram_parameter("out", [M, N_LOC], BF16, isOutput=True)

    AND = mybir.AluOpType.bitwise_and
    LSR = mybir.AluOpType.logical_shift_right
    MUL = mybir.AluOpType.mult

    with tile.TileContext(nc) as tc:
        from contextlib import ExitStack

        with ExitStack() as ctx:
            big = ctx.enter_context(tc.tile_pool(name="big", bufs=1))
            xT = big.tile([128, KT, M], BF16)        # x (k on partitions)
            W = big.tile([128, KT, NEXT], BF16)      # dequant + one-hot cols
            qw_sb = big.tile([128, KT, NP], I32)     # packed qweight, resident

            consts = ctx.enter_context(tc.tile_pool(name="consts", bufs=1))
            qz_sb = consts.tile([G, NP], I32)
            znib = consts.tile([G, NB, NP], I32)
            sps = [consts.tile([G, PH[p][1] - PH[p][0]], BF16, name=f"sps{p}")
                   for p in range(3)]
            Bs = [consts.tile([G, PH[p][1] - PH[p][0]], BF16, name=f"Bs{p}")
                  for p in range(3)]                 # +(z*s) per phase slice
            xsp = consts.tile([128, MT, 128], BF16)  # -xsum staging (padded)
            xsT = consts.tile([128, MT, 128], BF16)  # transposed -xsum per m

            wprep = ctx.enter_context(tc.tile_pool(name="wprep", bufs=2))
            opool = ctx.enter_context(tc.tile_pool(name="oout", bufs=3))

            # priming: first qweight chunk + first scale slice lead the
            # scalar ring; one-hot identity + first x chunk lead sync.
            def qw_load_range(k0, k1):
                nc.scalar.dma_start(
                    out=qw_sb[:, k0:k1, :],
                    in_=qw_d[:, k0:k1, :],
                )

            def x_load(c):
                if c == 0:
                    # kt0 alone: the first matmul's lhsT without waiting for
                    # a full 1MB chunk to clear the queues
                    nc.sync.dma_start(out=xT[:, 0:1, :], in_=xt_d[:, 0:1, :])
                    nc.sync.dma_start(
                        out=xT[:, 1:XCH, :], in_=xt_d[:, 1:XCH, :]
                    )
                else:
                    nc.sync.dma_start(
                        out=xT[:, c * XCH:(c + 1) * XCH, :],
                        in_=xt_d[:, c * XCH:(c + 1) * XCH, :],
                    )

            def dequant_pair(ph, j, s2_eng=None):
                plo, phi, wlo, _ = PH[ph]
                w = phi - plo
                pieces = PH_PIECES[ph]
                pw = sum(c1 - c0 for _, c0, c1 in pieces)
                kt = 2 * j
                nib = wprep.tile([128, 2, 256], I32, name="nib", tag="nib",
                                 bufs=2)
                off = 0
                for t, c0, c1 in pieces:
                    nc.vector.tensor_scalar(
                        out=nib[:, :, off:off + (c1 - c0)],
                        in0=qw_sb[:, kt:kt + 2, c0:c1],
                        scalar1=PAIR_SHIFTS[t], scalar2=PAIR_MASK,
                        op0=LSR, op1=AND,
                    )
                    off += c1 - c0
                # scale rows kt, kt+1 of this phase's slice: contiguous
                # single-segment 2*w*2B run per partition (own replica)
                s2 = wprep.tile([128, 2, 512], BF16, name="s2", tag="sbc",
                                bufs=6)
                rep = sp_d[ph][0]  # [G, w] view of replica 0
                (s2_eng or nc.sync).dma_start(
                    out=s2[:, :, 0:w],
                    in_=bass.AP(
                        tensor=rep.tensor,
                        offset=rep.offset + kt * w,
                        ap=[[G * w, 128], [w, 2], [1, w]],
                    ),
                )
                nib16 = nib.bitcast(I16)  # [128, 2, 512]
                nc.vector.tensor_tensor(
                    out=W[:, kt:kt + 2, wlo:wlo + 2 * pw],
                    in0=nib16[:, :, 0:2 * pw],
                    in1=s2[:, :, 0:w],
                    op=MUL,
                )

            def zb_prep():
                for t in range(NB):
                    nc.vector.tensor_scalar(
                        out=znib[:, t, :], in0=qz_sb,
                        scalar1=PAIR_SHIFTS[t], scalar2=PAIR_MASK,
                        op0=LSR, op1=AND,
                    )
                z16 = znib.bitcast(I16).rearrange("p a b -> p (a b)")
                for p in range(3):
                    plo, phi = PH[p][0], PH[p][1]
                    nc.vector.tensor_tensor(
                        out=Bs[p], in0=z16[:, plo:phi], in1=sps[p], op=MUL
                    )

            # one-hot columns: W[p, kt, 480+g] = (g == kt).  DMA into a
            # contiguous staging tile (128x2KB descriptors), then one DVE
            # copy scatters it into W's strided column block -- a direct
            # strided DMA would be 4096x64B descriptors and gate the PE.
            ohc = consts.tile([128, G, G], BF16)
            nc.sync.dma_start(out=ohc, in_=oh_d[:, :, :])
            nc.vector.tensor_copy(W[:, :, 480:512], ohc)
            nc.vector.memset(xsp, 0.0)

            pb = ctx.enter_context(
                tc.tile_pool(name="pb", bufs=1, space="PSUM")
            )

            qw_load_range(0, 2)   # pair 0 alone: unblocks the first dequant
            qw_load_range(2, 8)   # right behind on the same ring (FIFO)
            # preload the activation table used by scale=-1 copies so the
            # 1.3us lazy ACT_TABLE_LOAD doesn't land on a phase boundary
            nc.scalar.activation(
                xsp[:, 0, 0:1], xsp[:, 0, 0:1],
                mybir.ActivationFunctionType.Copy, scale=-1.0,
            )
            nc.scalar.dma_start(out=qz_sb, in_=qz_d[:, :])
            for p in range(3):
                nc.scalar.dma_start(out=sps[p], in_=sp_d[p][0])
            with tc.tile_wait_until(0.008):
                for c in range(1, KT // QCH):
                    qw_load_range(c * QCH, (c + 1) * QCH)
            zb_prep()

            def drain_m(ph, m, ps):
                plo, phi, _, _ = PH[ph]
                width = phi - plo
                # zero-point correction for this phase's columns; lhsT is
                # the quadrant-aligned 32-row slice of the grouped xsT
                if not _NO_CORR:
                    nc.tensor.matmul(
                        ps[:, 0:width],
                        lhsT=xsT[0:G, m, :],
                        rhs=Bs[ph],
                        start=False, stop=True,
                        skip_group_check=True,
                    )
                osb = opool.tile([128, 512], BF16, name="osb", tag="osb")
                if ph == 2 and m == MT - 1:
                    h = width // 2
                    nc.vector.tensor_copy(osb[:, 0:h], ps[:, 0:h])
                    nc.sync.dma_start(
                        out=out_d[m * 128:(m + 1) * 128, plo:plo + h],
                        in_=osb[:, 0:h],
                    )
                    nc.vector.tensor_copy(osb[:, h:width], ps[:, h:width])
                    nc.sync.dma_start(
                        out=out_d[m * 128:(m + 1) * 128, plo + h:phi],
                        in_=osb[:, h:width],
                    )
                else:
                    nc.vector.tensor_copy(osb[:, 0:width], ps[:, 0:width])
                    nc.sync.dma_start(
                        out=out_d[m * 128:(m + 1) * 128, plo:phi],
                        in_=osb[:, 0:width],
                    )

            def mk_ps(ph, m):
                return pb.tile([128, 512], F32, name=f"ps_{ph}_{m}",
                               tag=f"ps{m}", bufs=1)

            # ---- phase 0: kt-outer / m-inner (matches dequant+x pace) ----
            ps0 = [mk_ps(0, m) for m in range(MT)]
            for kt in range(KT):
                if kt % 2 == 0:
                    dequant_pair(0, kt // 2)
                if kt % XCH == 0:
                    x_load(kt // XCH)
                for m in range(MT):
                    nc.tensor.matmul(
                        ps0[m][:, 0:512],
                        lhsT=xT[:, kt, m * 128:(m + 1) * 128],
                        rhs=W[:, kt, 0:512],
                        start=(kt == 0), stop=False,
                        skip_group_check=True,
                    )
            # phase 1/2 dequant prefetch (issued under timeline pins; runs
            # during phase 0 on DVE + sync-ring slack)
            for ph in (1, 2):
                with tc.tile_wait_until(PH_PIN[ph]):
                    for j in range(KT // 2):
                        dequant_pair(ph, j)
            # phase-0 drains, fully interleaved per m-tile: m0's chain
            # gates phase 1's start; later m-tiles have a kt-sweep of slack
            for m in range(MT):
                nc.scalar.activation(
                    xsp[:, m, 0:G], ps0[m][:, 480:512],
                    mybir.ActivationFunctionType.Copy, scale=-1.0,
                )
                nc.scalar.dma_start(
                    out=xsT[:, m, :], in_=xsp[:, m, :], transpose=True
                )
                drain_m(0, m, ps0[m])

            # ---- phases 1, 2: m-outer / kt-inner (W fully resident) ----
            # each phase boundary gates on a single m-tile's drain and the
            # drains spread one per kt-sweep instead of piling up
            for ph in (1, 2):
                wlo, wwidth = PH[ph][2], PH[ph][3]
                for m in range(MT):
                    ps = mk_ps(ph, m)
                    for kt in range(KT):
                        nc.tensor.matmul(
                            ps[:, 0:wwidth],
                            lhsT=xT[:, kt, m * 128:(m + 1) * 128],
                            rhs=W[:, kt, wlo:wlo + wwidth],
                            start=(kt == 0), stop=False,
                            skip_group_check=True,
                        )
                    drain_m(ph, m, ps)

    return nc


def _get_nc():
    global _cached_nc
    if _cached_nc is None:
        _cached_nc = _build()
    return _cached_nc


def kernel(x, qweight, scales, qzeros):
    global LAST_EXEC_NS, LAST_TRACE

    x = np.asarray(x, dtype=np.float32)
    # partition-major x: [128, KT, M]
    x_t = np.ascontiguousarray(
        x.T.astype(ml_dtypes.bfloat16).reshape(KT, 128, M).transpose(1, 0, 2)
    )
    qweight = np.asarray(qweight, dtype=np.int32)
    scales = np.asarray(scales, dtype=np.float32)
    qzeros = np.asarray(qzeros, dtype=np.int32)
    oh = np.ascontiguousarray(
        np.broadcast_to(np.eye(G, dtype=ml_dtypes.bfloat16)[None], (128, G, G))
    )

    in_maps = []
    for c in range(NCORES):
        # partition-major qweight: qw_pm[p, a, :] = qweight[a*128 + p, cols]
        qw_c = qweight[:, c * NP:(c + 1) * NP]
        qw_pm = np.ascontiguousarray(
            qw_c.reshape(KT, 128, NP).transpose(1, 0, 2)
        )
        qz_c = np.ascontiguousarray(qzeros[:, c * NP:(c + 1) * NP])
        s_c = scales[:, c * N_LOC:(c + 1) * N_LOC]
        # pair-block permutation: dest[g, 344*t + 2*cc + r] = s[g, 8*cc + 2*t + r]
        s_perm = np.ascontiguousarray(
            s_c.reshape(G, NP, 4, 2).transpose(0, 2, 1, 3).reshape(G, N_LOC)
        ).astype(ml_dtypes.bfloat16)
        im = {"xt": x_t, "qw": qw_pm, "qz": qz_c, "oh": oh}
        for p in range(3):
            plo, phi = PH[p][0], PH[p][1]
            im[f"sp{p}"] = np.ascontiguousarray(
                np.broadcast_to(s_perm[None, :, plo:phi],
                                (SREP, G, phi - plo))
            )
        in_maps.append(im)

    trace = os.environ.get("AWQ_KERNEL_TRACE", "0") == "1"
    if trace:
        _install_ntff_shim()

    nc = _get_nc()
    res = run_bass_kernel_spmd(
        nc, in_maps, core_ids=list(range(NCORES)), trace=trace
    )
    LAST_EXEC_NS = res.exec_time_ns
    if res.instructions_and_trace is not None:
        LAST_TRACE = res.instructions_and_trace[1]

    # undo the pair-block column permutation on the host (pure reindex)
    return np.concatenate(
        [np.asarray(res.results[i]["out"]).astype(np.float32)[:, _INV]
         for i in range(NCORES)],
        axis=1,
    )
